# revision 1
# baseline (speedup 1.0000x reference)
"""CSWin block (B=8,H=W=56,C=256) on 8 trn2 NeuronCores, data-parallel over batch.

Layout strategy (per core, one image of 3136 tokens):
  - LayerNorms computed token-major (tokens on partitions) -> per-token stats are
    per-partition scalars (native tensor_scalar), then PE-transposed to
    channel-major for the matmuls.
  - All on-chip activations feeding matmuls are channel-major [C, T] so matmuls
    contract along partitions with zero transposes between layers.
  - Branch 0 tensors are stored in W-major token order, branch 1 in H-major, so
    every CSWin stripe window is a contiguous 392-token slice for both branches.
  - Attention computes S^T = K^T Q per window (softmax dim on partitions is
    avoided entirely: exp on ACT, row sums come free as a ones-column in the
    transposed-V operand of the O matmul), O accumulates channel-major.
  - LePE depthwise conv runs on the TensorEngine as 9 diag-matrix matmuls with
    shifted access patterns, accumulated in PSUM during the (PSUM-idle) QKV phase.
  - dtypes: bf16 for q/k/v/Et/Vt/lepe/mlp-h + their weights, float32r (TF32-ish)
    for att/proj, fp32 for LN stats, residuals and PSUM.
"""

import sys

sys.path.insert(0, "/opt/trn_rl_repo")

import numpy as np
from contextlib import ExitStack

import concourse.bacc as bacc
import concourse.tile as tile
import concourse.mybir as mybir
from concourse.bass_utils import run_bass_kernel_spmd
from concourse.masks import make_identity

F32 = mybir.dt.float32
F32R = mybir.dt.float32r
BF16 = mybir.dt.bfloat16
AF = mybir.ActivationFunctionType
OP = mybir.AluOpType

B, HH, WW, C = 8, 56, 56, 256
T = HH * WW              # 3136 tokens
NW = 8                   # windows per branch
L = 392                  # tokens per window
NH = 4                   # heads per branch
HD = 32                  # head dim
MC = 98                  # m-chunk (window tokens / 4)
TK = 112                 # token chunk for token-major phases
NTOK = T // TK           # 28
EPS = 1e-5
SM_SCALE = float(HD) ** -0.5

_CACHE = {}


def _rhs_qkv(ln1, bi, kch, t):
    """ln1[kch] is channel-major H-major-token [128, 3136]; returns the rhs AP for
    output N-tile t (392 tokens) in the branch's native token order."""
    if bi == 0:  # W-major order: iterate (w, h)
        return ln1[kch].rearrange("p (h w) -> p w h", h=HH)[:, 7 * t:7 * t + 7, :]
    return ln1[kch][:, L * t:L * (t + 1)]


def _build():
    nc = bacc.Bacc("TRN2", target_bir_lowering=False, debug=False,
                   enable_asserts=False, num_devices=8)

    x_d = nc.dram_tensor("x", [T, C], F32, kind="ExternalInput").ap()
    out_d = nc.dram_tensor("out", [T, C], F32, kind="ExternalOutput").ap()
    wqkv_d = nc.dram_tensor("w_qkv", [C, 3 * C], F32, kind="ExternalInput").ap()
    wproj_d = nc.dram_tensor("w_proj", [C, C], F32, kind="ExternalInput").ap()
    bproj_d = nc.dram_tensor("b_proj", [C], F32, kind="ExternalInput").ap()
    g1_d = nc.dram_tensor("gamma1", [C], F32, kind="ExternalInput").ap()
    be1_d = nc.dram_tensor("beta1", [C], F32, kind="ExternalInput").ap()
    g2_d = nc.dram_tensor("gamma2", [C], F32, kind="ExternalInput").ap()
    be2_d = nc.dram_tensor("beta2", [C], F32, kind="ExternalInput").ap()
    wfc1_d = nc.dram_tensor("w_fc1", [C, 4 * C], F32, kind="ExternalInput").ap()
    bfc1_d = nc.dram_tensor("b_fc1", [4 * C], F32, kind="ExternalInput").ap()
    wfc2_d = nc.dram_tensor("w_fc2", [4 * C, C], F32, kind="ExternalInput").ap()
    bfc2_d = nc.dram_tensor("b_fc2", [C], F32, kind="ExternalInput").ap()
    lw_d = [nc.dram_tensor("lepe_w0", [128, 9], F32, kind="ExternalInput").ap(),
            nc.dram_tensor("lepe_w1", [128, 9], F32, kind="ExternalInput").ap()]
    lb_d = [nc.dram_tensor("lepe_b0", [128], F32, kind="ExternalInput").ap(),
            nc.dram_tensor("lepe_b1", [128], F32, kind="ExternalInput").ap()]

    with tile.TileContext(nc) as tc:
        with ExitStack() as ctx:
            _emit(nc, tc, ctx, locals())
    nc.compile()
    return nc


def _emit(nc, tc, ctx, d):
    x_d, out_d = d["x_d"], d["out_d"]
    lw_d, lb_d = d["lw_d"], d["lb_d"]

    pp = ctx.enter_context(tc.tile_pool(name="pp", bufs=1))
    psmall = ctx.enter_context(tc.tile_pool(name="psmall", bufs=1))

    # ---------------- persistent tensors ----------------
    xtok = pp.tile([TK, NTOK, C], F32, name="xtok")
    x2res = pp.tile([TK, NTOK, C], F32, name="x2res")
    wqkv_b = pp.tile([128, 2, 3 * C], BF16, name="wqkv_b")
    wproj_r = pp.tile([128, 2, C], F32R, name="wproj_r")
    wfc1_b = pp.tile([128, 2, 4 * C], BF16, name="wfc1_b")
    wfc2_b = pp.tile([128, 8, C], BF16, name="wfc2_b")
    lwb = pp.tile([128, 2, 9], BF16, name="lwb")
    g1s = psmall.tile([128, 2], F32, name="g1s")
    b1s = psmall.tile([128, 2], F32, name="b1s")
    g2s = psmall.tile([128, 2], F32, name="g2s")
    b2s = psmall.tile([128, 2], F32, name="b2s")
    bfc1s = psmall.tile([128, 8], F32, name="bfc1s")
    lbs = psmall.tile([128, 2], F32, name="lbs")
    bproj_tm = psmall.tile([TK, C], F32, name="bproj_tm")
    bfc2_tm = psmall.tile([TK, C], F32, name="bfc2_tm")
    id_r = psmall.tile([128, 128], F32R, name="id_r")
    id_b = psmall.tile([128, 128], BF16, name="id_b")
    mv1 = psmall.tile([TK, NTOK, 2], F32, name="mv1")
    rstd1 = psmall.tile([TK, NTOK], F32, name="rstd1")
    std1 = psmall.tile([TK, NTOK], F32, name="std1")
    mv2 = psmall.tile([TK, NTOK, 2], F32, name="mv2")
    rstd2 = psmall.tile([TK, NTOK], F32, name="rstd2")
    std2 = psmall.tile([TK, NTOK], F32, name="std2")
    eps_t = psmall.tile([TK, 1], F32, name="eps_t")
    nc.vector.memset(eps_t, EPS)

    # ---------------- phase A: loads + weight conversion ----------------
    with tc.tile_pool(name="stg", bufs=1) as stg:
        wqkv_f = stg.tile([128, 2, 3 * C], F32, name="wqkv_f")
        wproj_f = stg.tile([128, 2, C], F32, name="wproj_f")
        wfc1_f = stg.tile([128, 2, 4 * C], F32, name="wfc1_f")
        wfc2_f = stg.tile([128, 8, C], F32, name="wfc2_f")
        lw_f = stg.tile([128, 2, 9], F32, name="lw_f")

        nc.sync.dma_start(wqkv_f, d["wqkv_d"].rearrange("(a p) n -> p a n", p=128))
        nc.sync.dma_start(wproj_f, d["wproj_d"].rearrange("(a p) n -> p a n", p=128))
        nc.sync.dma_start(wfc1_f, d["wfc1_d"].rearrange("(a p) n -> p a n", p=128))
        nc.sync.dma_start(wfc2_f, d["wfc2_d"].rearrange("(a p) n -> p a n", p=128))
        for bi in range(2):
            nc.sync.dma_start(lw_f[:, bi, :], lw_d[bi])
            nc.sync.dma_start(lbs[:, bi:bi + 1], lb_d[bi].unsqueeze(1))
        nc.sync.dma_start(g1s, d["g1_d"].rearrange("(a p) -> p a", p=128))
        nc.sync.dma_start(b1s, d["be1_d"].rearrange("(a p) -> p a", p=128))
        nc.sync.dma_start(g2s, d["g2_d"].rearrange("(a p) -> p a", p=128))
        nc.sync.dma_start(b2s, d["be2_d"].rearrange("(a p) -> p a", p=128))
        nc.sync.dma_start(bfc1s, d["bfc1_d"].rearrange("(a p) -> p a", p=128))
        nc.sync.dma_start(bproj_tm, d["bproj_d"].unsqueeze(0).broadcast_to([TK, C]))
        nc.sync.dma_start(bfc2_tm, d["bfc2_d"].unsqueeze(0).broadcast_to([TK, C]))

        nc.vector.tensor_copy(wqkv_b, wqkv_f)
        nc.vector.tensor_copy(wproj_r, wproj_f)
        nc.vector.tensor_copy(wfc1_b, wfc1_f)
        nc.vector.tensor_copy(wfc2_b, wfc2_f)
        nc.vector.tensor_copy(lwb, lw_f)
        id_f = stg.tile([128, 128], F32, name="id_f")
        make_identity(nc, id_f)
        nc.vector.tensor_copy(id_r, id_f)
        nc.vector.tensor_copy(id_b, id_f)

        # input: token-major [112, 28, 256]; 4 DMAs for queue parallelism
        xsrc = x_d.rearrange("(i p) c -> p i c", p=TK)
        for j in range(4):
            nc.sync.dma_start(xtok[:, 7 * j:7 * j + 7, :], xsrc[:, 7 * j:7 * j + 7, :])

    # ---------------- LN (token-major) helper ----------------
    def layer_norm(src, mv, stdt, rstd, gs, bs, dst, lnp, tpp):
        """src: [TK, NTOK, C] f32; dst: 2 channel-major [128, T] bf16 tiles."""
        for i in range(NTOK):
            st = lnp.tile([TK, 6], F32, name="bnst", tag="bnst")
            nc.vector.bn_stats(st, src[:, i, :])
            nc.vector.bn_aggr(mv[:, i, :], st)
        nc.scalar.activation(stdt, mv[:, :, 1], AF.Sqrt, bias=eps_t)
        nc.vector.reciprocal(rstd, stdt)
        for g in range(7):
            lnt = []
            for j in range(4):
                i = 4 * g + j
                lt = lnp.tile([TK, C], F32R, name="lnt", tag="lnt")
                nc.vector.tensor_scalar(
                    out=lt, in0=src[:, i, :],
                    scalar1=mv[:, i, 0:1], scalar2=rstd[:, i:i + 1],
                    op0=OP.subtract, op1=OP.mult)
                lnt.append(lt)
            for c in range(2):
                tp = tpp.tile([128, 4 * TK], F32R, name="lntp", tag="lntp")
                for j in range(4):
                    nc.tensor.transpose(tp[:, TK * j:TK * (j + 1)],
                                        lnt[j][:, 128 * c:128 * (c + 1)],
                                        id_r[0:TK, 0:TK])
                nc.scalar.activation(dst[c][:, 4 * TK * g:4 * TK * (g + 1)], tp,
                                     AF.Identity, bias=bs[:, c:c + 1],
                                     scale=gs[:, c:c + 1])

    # ---------------- attention-lifetime tensors ----------------
    actx = ExitStack()
    attn_pool = actx.enter_context(tc.tile_pool(name="attn_pool", bufs=1))
    qc = [attn_pool.tile([128, T], BF16, name=f"qc{b}") for b in range(2)]
    kc = [attn_pool.tile([128, T], BF16, name=f"kc{b}") for b in range(2)]
    vtb = [attn_pool.tile([MC, NW, 4, 4 * 33], BF16, name=f"vtb{b}") for b in range(2)]
    lepe_sb = [attn_pool.tile([128, T], BF16, name=f"lepe{b}") for b in range(2)]
    att_pool = actx.enter_context(tc.tile_pool(name="att_pool", bufs=1))
    att = [att_pool.tile([128, T], F32R, name=f"att{b}") for b in range(2)]

    # Optional in-NEFF repetition loop for wall-clock timing (BASS_KERNEL_ITERS>1)
    import os as _os
    _iters = int(_os.environ.get("BASS_KERNEL_ITERS", "1"))
    _skip = _os.environ.get("BASS_KERNEL_SKIP", "")
    loop_cm = tc.For_i(0, _iters, 1) if _iters > 1 else None
    if loop_cm is not None:
        ctx.enter_context(loop_cm)

    # ---------------- phase B: LN1 ----------------
    with tc.tile_pool(name="ln1cm", bufs=1) as lncm:
        ln1 = [lncm.tile([128, T], BF16, name=f"ln1_{c}") for c in range(2)]
        with tc.tile_pool(name="lnp1", bufs=8) as lnp, \
             tc.tile_pool(name="tpp1", bufs=2, space="PSUM") as tpp:
            layer_norm(xtok, mv1, std1, rstd1, g1s, b1s, ln1, lnp, tpp)

        # ---------------- phase C: QKV + V-transpose + LePE ----------------
        with tc.tile_pool(name="vcp", bufs=1) as vcp, \
             tc.tile_pool(name="qkvps", bufs=2, space="PSUM") as qkvps, \
             tc.tile_pool(name="vtps", bufs=2, space="PSUM") as vtps, \
             tc.tile_pool(name="lpps", bufs=2, space="PSUM") as lpps, \
             tc.tile_pool(name="dgp", bufs=2) as dgp:
            vc = [vcp.tile([128, T], BF16, name=f"vc{b}") for b in range(2)]
            # chunk name -> (branch, wqkv col range, destination)
            chunks = [("v0", 0, 512, vc[0]), ("v1", 1, 640, vc[1]),
                      ("q0", 0, 0, qc[0]), ("k0", 0, 256, kc[0]),
                      ("q1", 1, 128, qc[1]), ("k1", 1, 384, kc[1])]
            if "C" in _skip:
                chunks = []
            for ci, (nm, bi, c0, dst) in enumerate(chunks):
                for t2 in range(NW // 2):
                    pt = qkvps.tile([128, 1024], F32, name="qkvt", tag="qkvt")
                    for half in range(2):
                        t = 2 * t2 + half
                        for kch in range(2):
                            nc.tensor.matmul(pt[:, 512 * half:512 * half + L],
                                             wqkv_b[:, kch, c0:c0 + 128],
                                             _rhs_qkv(ln1, bi, kch, t),
                                             start=(kch == 0), stop=(kch == 1))
                    dstap = dst[:, 2 * L * t2:2 * L * (t2 + 1)].rearrange(
                        "p (a x) -> p a x", a=2)
                    srcap = pt.rearrange("p (a x) -> p a x", a=2)[:, :, 0:L]
                    if (t2 + ci) % 2 == 0:
                        nc.scalar.copy(dstap, srcap)
                    else:
                        nc.vector.tensor_copy(dstap, srcap)

            for bi in (range(2) if "C" not in _skip else ()):
                # V transposes -> [token, ch] with interleaved ones columns
                for w2 in range(NW // 2):
                    vt = vtps.tile([MC, 2, 512], BF16, name="vtt", tag="vtt")
                    for half in range(2):
                        w = 2 * w2 + half
                        for m in range(4):
                            nc.tensor.transpose(
                                vt[:, half, 128 * m:128 * (m + 1)],
                                vc[bi][:, L * w + MC * m:L * w + MC * (m + 1)],
                                id_b)
                    dstv = vtb[bi][:, 2 * w2:2 * w2 + 2].rearrange(
                        "p a m (h e) -> p a m h e", e=33)
                    nc.vector.tensor_copy(
                        dstv[:, :, :, :, 0:32],
                        vt.rearrange("p a (m h e) -> p a m h e", m=4, h=4))
                    for half in range(2):
                        nc.vector.memset(dstv[:, half, :, :, 32:33], 1.0)
                # LePE: 9 diag matmuls per window
                dg = dgp.tile([128, 9, 128], BF16, name="diag", tag="diag")
                for tx in (-1, 0, 1):
                    for ty in (-1, 0, 1):
                        wi = ((ty + 1) * 3 + (tx + 1)) if bi == 0 else ((tx + 1) * 3 + (ty + 1))
                        nc.gpsimd.affine_select(
                            out=dg[:, wi, :],
                            in_=lwb[:, bi, wi:wi + 1].broadcast_to([128, 128]),
                            compare_op=OP.is_equal, fill=0.0, base=0,
                            pattern=[[-1, 128]], channel_multiplier=1)
                taps = [(0, 0)] + [(tx, ty) for tx in (-1, 0, 1) for ty in (-1, 0, 1)
                                   if (tx, ty) != (0, 0)]
                for w in range(NW):
                    lp = lpps.tile([128, L], F32, name="lpt", tag="lpt")
                    lpv = lp.rearrange("p (x y) -> p x y", x=7)
                    vcv = vc[bi][:, L * w:L * (w + 1)].rearrange("p (x y) -> p x y", x=7)
                    for ti, (tx, ty) in enumerate(taps):
                        wi = ((ty + 1) * 3 + (tx + 1)) if bi == 0 else ((tx + 1) * 3 + (ty + 1))
                        xo0, xo1 = max(0, -tx), 7 - max(0, tx)
                        yo0, yo1 = max(0, -ty), HH - max(0, ty)
                        nc.tensor.matmul(
                            lpv[:, xo0:xo1, yo0:yo1], dg[:, wi, :],
                            vcv[:, xo0 + tx:xo1 + tx, yo0 + ty:yo1 + ty],
                            start=(ti == 0), stop=(ti == 8))
                    nc.scalar.activation(lepe_sb[bi][:, L * w:L * (w + 1)], lp,
                                         AF.Identity, bias=lbs[:, bi:bi + 1])

    # ---------------- phase D: windowed attention ----------------
    with tc.tile_pool(name="etp", bufs=2) as etp, \
         tc.tile_pool(name="sps", bufs=1, space="PSUM") as sps, \
         tc.tile_pool(name="ops", bufs=1, space="PSUM") as ops, \
         tc.tile_pool(name="rrp", bufs=2) as rrp, \
         tc.tile_pool(name="rbp", bufs=2) as rbp:
        for bi in (range(2) if "D" not in _skip else ()):
            for w in range(NW):
                et = etp.tile([MC, 4, 4, L], BF16, name="et", tag="et")
                ohalf = [ops.tile([33, 1024], F32, name=f"opst{z}", tag=f"opst{z}")
                         for z in range(2)]
                for m in range(4):
                    sp = sps.tile([MC, 2048], F32, name="spst", tag="spst")
                    for h in range(NH):
                        nc.tensor.matmul(
                            sp[:, 512 * h:512 * h + L],
                            kc[bi][32 * h:32 * (h + 1), L * w + MC * m:L * w + MC * (m + 1)],
                            qc[bi][32 * h:32 * (h + 1), L * w:L * (w + 1)],
                            start=True, stop=True, tile_position=(32 * h, 0))
                    nc.scalar.activation(
                        et[:, m, :, :],
                        sp.rearrange("p (h x) -> p h x", h=4)[:, :, 0:L],
                        AF.Exp, scale=SM_SCALE)
                    for h in range(NH):
                        nc.tensor.matmul(
                            ohalf[h // 2][:, 512 * (h % 2):512 * (h % 2) + L],
                            vtb[bi][:, w, m, 33 * h:33 * (h + 1)],
                            et[:, m, h, :],
                            start=(m == 0), stop=(m == 3))
                # att[0] is stored H-major (so proj lhsT slices are contiguous);
                # branch-0 windows therefore write through a strided [w, h] view.
                if bi == 0:
                    attw = att[0].rearrange("p (h w) -> p w h", h=HH)[:, 7 * w:7 * w + 7, :]
                else:
                    attw = att[1][:, L * w:L * (w + 1)].rearrange("p (a b) -> p a b", a=7)
                lpw = lepe_sb[bi][:, L * w:L * (w + 1)].rearrange("p (a b) -> p a b", a=7)
                for z in range(2):
                    op_ = ohalf[z]
                    rr = rrp.tile([1, 2, L], F32, name="rr", tag="rr")
                    nc.vector.reciprocal(
                        rr, op_[32:33, :].rearrange("p (h x) -> p h x", h=2)[:, :, 0:L])
                    rb = rbp.tile([32, 2, L], F32, name="rb", tag="rb")
                    nc.gpsimd.partition_broadcast(rb, rr)
                    for hh in range(2):
                        h = 2 * z + hh
                        nc.vector.tensor_tensor(
                            out=attw[32 * h:32 * (h + 1)],
                            in0=op_[0:32, 512 * hh:512 * hh + L].rearrange(
                                "p (a b) -> p a b", a=7),
                            in1=rb[:, hh, :].rearrange("p (a b) -> p a b", a=7),
                            op=OP.mult)
                nc.vector.tensor_tensor(out=attw, in0=attw, in1=lpw, op=OP.add)

    # ---------------- phase E: proj + residual (token-major out) ----------------
    # xtok += b_proj (broadcast) so the proj evict is a single fused add
    if "E" not in _skip:
        nc.gpsimd.tensor_tensor(
            out=xtok, in0=xtok,
            in1=bproj_tm.unsqueeze(1).broadcast_to([TK, NTOK, C]),
            op=OP.add)
    with tc.tile_pool(name="prps", bufs=4, space="PSUM") as prps:
        for i2 in (range(NTOK // 2) if "E" not in _skip else ()):
            pt = prps.tile([TK, 2, C], F32, name="prt", tag="prt")
            for half in range(2):
                i = 2 * i2 + half
                for kch in range(2):
                    lhs = att[kch][:, TK * i:TK * (i + 1)]
                    nc.tensor.matmul(pt[:, half, :], lhs, wproj_r[:, kch, :],
                                     start=(kch == 0), stop=(kch == 1))
            nc.vector.scalar_tensor_tensor(
                out=x2res[:, 2 * i2:2 * i2 + 2, :], in0=pt, scalar=1.0,
                in1=xtok[:, 2 * i2:2 * i2 + 2, :], op0=OP.mult, op1=OP.add)

    actx.close()

    # ---------------- phase F: LN2 + MLP ----------------
    with tc.tile_pool(name="lnp2", bufs=8) as lnp2, \
         tc.tile_pool(name="tpp2", bufs=2, space="PSUM") as tpp2, \
         tc.tile_pool(name="mlp", bufs=1) as mlp:
        ln2 = [mlp.tile([128, T], BF16, name=f"ln2_{c}") for c in range(2)]
        if "F" not in _skip:
            layer_norm(x2res, mv2, std2, rstd2, g2s, b2s, ln2, lnp2, tpp2)

        h_sb = mlp.tile([128, 8, T], BF16, name="h_sb")
        with tc.tile_pool(name="f1ps", bufs=3, space="PSUM") as f1ps:
            for m8 in (range(8) if "G" not in _skip else ()):
                for tp2 in range(NW // 2):
                    pt = f1ps.tile([128, 1024], F32, name="f1t", tag="f1t")
                    for half in range(2):
                        t = 2 * tp2 + half
                        for kch in range(2):
                            nc.tensor.matmul(pt[:, 512 * half:512 * half + L],
                                             wfc1_b[:, kch, 128 * m8:128 * (m8 + 1)],
                                             ln2[kch][:, L * t:L * (t + 1)],
                                             start=(kch == 0), stop=(kch == 1))
                    nc.scalar.activation(
                        h_sb[:, m8, 2 * L * tp2:2 * L * (tp2 + 1)].rearrange(
                            "p (a x) -> p a x", a=2),
                        pt.rearrange("p (a x) -> p a x", a=2)[:, :, 0:L],
                        AF.Gelu, bias=bfc1s[:, m8:m8 + 1])

        # x2res += b_fc2 (broadcast) after LN2 consumed raw x2res
        if "G" not in _skip:
            nc.gpsimd.tensor_tensor(
                out=x2res, in0=x2res,
                in1=bfc2_tm.unsqueeze(1).broadcast_to([TK, NTOK, C]),
                op=OP.add)
        with tc.tile_pool(name="f2ps", bufs=4, space="PSUM") as f2ps, \
             tc.tile_pool(name="otp", bufs=4) as otp:
            for i2 in (range(NTOK // 2) if "G" not in _skip else ()):
                pt = f2ps.tile([TK, 2, C], F32, name="f2t", tag="f2t")
                for half in range(2):
                    i = 2 * i2 + half
                    for k8 in range(8):
                        nc.tensor.matmul(pt[:, half, :], h_sb[:, k8, TK * i:TK * (i + 1)],
                                         wfc2_b[:, k8, :],
                                         start=(k8 == 0), stop=(k8 == 7))
                ot = otp.tile([TK, 2, C], F32, name="ot", tag="ot")
                nc.vector.scalar_tensor_tensor(
                    out=ot, in0=pt, scalar=1.0, in1=x2res[:, 2 * i2:2 * i2 + 2, :],
                    op0=OP.mult, op1=OP.add)
                eng = nc.sync if i2 % 2 == 0 else nc.scalar
                eng.dma_start(
                    out_d[2 * TK * i2:2 * TK * (i2 + 1), :].rearrange(
                        "(a p) c -> p a c", p=TK),
                    ot)


def kernel(**inputs):
    if "nc" not in _CACHE:
        _CACHE["nc"] = _build()
    nc = _CACHE["nc"]

    x = np.asarray(inputs["x"], dtype=np.float32)          # [8, 56, 56, 256]
    base = {
        "w_qkv": np.asarray(inputs["w_qkv"], np.float32),
        "w_proj": np.asarray(inputs["w_proj"], np.float32),
        "b_proj": np.asarray(inputs["b_proj"], np.float32),
        "gamma1": np.asarray(inputs["gamma1"], np.float32),
        "beta1": np.asarray(inputs["beta1"], np.float32),
        "gamma2": np.asarray(inputs["gamma2"], np.float32),
        "beta2": np.asarray(inputs["beta2"], np.float32),
        "w_fc1": np.asarray(inputs["w_fc1"], np.float32),
        "b_fc1": np.asarray(inputs["b_fc1"], np.float32),
        "w_fc2": np.asarray(inputs["w_fc2"], np.float32),
        "b_fc2": np.asarray(inputs["b_fc2"], np.float32),
        "lepe_w0": np.asarray(inputs["lepe_w0"], np.float32).reshape(128, 9),
        "lepe_w1": np.asarray(inputs["lepe_w1"], np.float32).reshape(128, 9),
        "lepe_b0": np.asarray(inputs["lepe_b0"], np.float32),
        "lepe_b1": np.asarray(inputs["lepe_b1"], np.float32),
    }
    in_maps = [{**base, "x": np.ascontiguousarray(x[i].reshape(T, C))}
               for i in range(B)]
    import os
    trace = bool(int(os.environ.get("BASS_KERNEL_TRACE", "0")))
    res = run_bass_kernel_spmd(nc, in_maps, core_ids=list(range(B)), trace=trace)
    _CACHE["last_results"] = res
    out = np.stack([res.results[i]["out"] for i in range(B)])
    return out.reshape(B, HH, WW, C)


if __name__ == "__main__":
    rng = np.random.default_rng(0)
    ins = {
        "x": rng.standard_normal((B, HH, WW, C), dtype=np.float32),
        "gamma1": np.ones(C, np.float32), "beta1": np.zeros(C, np.float32),
        "w_qkv": rng.standard_normal((C, 3 * C), dtype=np.float32) * 0.02,
        "lepe_w0": rng.standard_normal((128, 1, 3, 3), dtype=np.float32) * 0.02,
        "lepe_b0": np.zeros(128, np.float32),
        "lepe_w1": rng.standard_normal((128, 1, 3, 3), dtype=np.float32) * 0.02,
        "lepe_b1": np.zeros(128, np.float32),
        "w_proj": rng.standard_normal((C, C), dtype=np.float32) * 0.02,
        "b_proj": np.zeros(C, np.float32),
        "gamma2": np.ones(C, np.float32), "beta2": np.zeros(C, np.float32),
        "w_fc1": rng.standard_normal((C, 4 * C), dtype=np.float32) * 0.02,
        "b_fc1": np.zeros(4 * C, np.float32),
        "w_fc2": rng.standard_normal((4 * C, C), dtype=np.float32) * 0.02,
        "b_fc2": np.zeros(C, np.float32),
    }
    o = kernel(**ins)
    print("ran:", o.shape, o.dtype, float(np.abs(o).max()))



# revision 30
# speedup vs baseline: 1.5142x; 1.5142x over previous
"""CSWin block (B=8,H=W=56,C=256) on 8 trn2 NeuronCores, data-parallel over batch.

v2.1 design:
  - Attention O computed TOKEN-major (out = E^T-chunk.T @ V_tok): each O
    matmul has free-size 33 (vs 392 channel-major); the softmax denominator
    is a static ones column in the token-major V tile. Normalize is one
    reciprocal + broadcast-multiply per q-chunk pair (PSUM -> SBUF bf16).
  - Per-window just-in-time pipeline: each slot emits its own QKV columns,
    V-transpose, LePE and S/exp/O, alternating branches (b1 two slots ahead
    since b0's W-major QKV reads need the full LN1). The attention stretch is
    exp(ACT)-bound; PE fills with the next slot's prep.
  - att transposed back to channel-major via PE transposes deferred to the
    proj phase; the evict fuses the LePE add.
  - gamma/beta folded into w_qkv / w_fc1 host-side (beta rows become
    per-channel biases fused into the psum evicts); b_proj folded into x
    host-side; b_fc2 added via a K=1 ones-row matmul in the fc2 accumulation.
  - LN applies (x-m)*rstd on ACT (bias/scale per-partition), stats on DVE,
    channel-major evicts on Pool/DVE; LN2 stats interleave with the proj
    residual; fc1 runs t-outer so fc2 pipelines behind it.
"""

import sys

sys.path.insert(0, "/opt/trn_rl_repo")

import numpy as np
from contextlib import ExitStack

import concourse.bacc as bacc
import concourse.tile as tile
import concourse.mybir as mybir
from concourse.bass_utils import run_bass_kernel_spmd
from concourse.masks import make_identity

F32 = mybir.dt.float32
F32R = mybir.dt.float32r
BF16 = mybir.dt.bfloat16
FP8 = mybir.dt.float8e4
PM = mybir.MatmulPerfMode
AF = mybir.ActivationFunctionType
OP = mybir.AluOpType

B, HH, WW, C = 8, 56, 56, 256
T = HH * WW              # 3136 tokens
NW = 8                   # windows per branch
L = 392                  # tokens per window
NH = 4                   # heads per branch
HD = 32                  # head dim
MC = 98                  # key-chunk (window tokens / 4) for S^T
QCS = [112, 112, 112, 56]  # q-chunks for token-major O (x-aligned for b0)
QCO = [0, 112, 224, 336]
TK = 112                 # token chunk for token-major phases
NTOK = T // TK           # 28
EPS = 1e-5
SM_SCALE = float(HD) ** -0.5

_CACHE = {}


def _rhs_qkv(ln1_cm, bi, t):
    """ln1_cm: channel-major H-major-token [128, 2, T]; kch-pair rhs AP for
    output N-tile t (392 tokens) in the branch's native token order."""
    if bi == 0:  # W-major order: iterate (w, h)
        return ln1_cm.rearrange(
            "p a (h w) -> p a w h", h=HH)[:, :, 7 * t:7 * t + 7, :]
    return ln1_cm[:, :, L * t:L * (t + 1)]


def _build():
    nc = bacc.Bacc("TRN2", target_bir_lowering=False, debug=False,
                   enable_asserts=False, num_devices=8)

    x_d = nc.dram_tensor("x", [T, C], F32, kind="ExternalInput").ap()
    out_d = nc.dram_tensor("out", [T, C], F32, kind="ExternalOutput").ap()
    wqkv_d = nc.dram_tensor("w_qkv", [C, 3 * C], F32, kind="ExternalInput").ap()
    wproj_d = nc.dram_tensor("w_proj", [C, C], F32, kind="ExternalInput").ap()
    qkvb_d = nc.dram_tensor("qkv_bias", [128, 6], F32, kind="ExternalInput").ap()
    wfc1_d = nc.dram_tensor("w_fc1", [C, 4 * C], F32, kind="ExternalInput").ap()
    fc1b_d = nc.dram_tensor("fc1_bias", [128, 8], F32, kind="ExternalInput").ap()
    wfc2_d = nc.dram_tensor("w_fc2", [4 * C, C], F32, kind="ExternalInput").ap()
    bfc2_d = nc.dram_tensor("b_fc2", [1, C], F32, kind="ExternalInput").ap()
    lw_d = [nc.dram_tensor("lepe_w0", [128, 9], F32, kind="ExternalInput").ap(),
            nc.dram_tensor("lepe_w1", [128, 9], F32, kind="ExternalInput").ap()]
    lb_d = [nc.dram_tensor("lepe_b0", [128], F32, kind="ExternalInput").ap(),
            nc.dram_tensor("lepe_b1", [128], F32, kind="ExternalInput").ap()]

    with tile.TileContext(nc) as tc:
        with ExitStack() as ctx:
            _emit(nc, tc, ctx, locals())
    nc.compile()
    return nc


def _emit(nc, tc, ctx, d):
    import os
    SKIP = os.environ.get("BASS_SKIP", "")
    NODR = bool(int(os.environ.get("BASS_NODR", "0")))
    DT8 = BF16 if NODR else FP8
    x_d, out_d = d["x_d"], d["out_d"]
    lw_d, lb_d = d["lw_d"], d["lb_d"]

    pp = ctx.enter_context(tc.tile_pool(name="pp", bufs=1))
    psmall = ctx.enter_context(tc.tile_pool(name="psmall", bufs=1))

    # ---------------- persistent tensors ----------------
    xtok = pp.tile([TK, NTOK, C], F32, name="xtok")
    x2res = pp.tile([TK, NTOK, C], F32, name="x2res")
    wqkv_b = pp.tile([128, 2, 3 * C], DT8, name="wqkv_b")
    wproj_b = pp.tile([128, 2, C], DT8, name="wproj_b")
    lwb = pp.tile([128, 2, 9], BF16, name="lwb")
    qkvbias = psmall.tile([128, 6], F32, name="qkvbias")
    fc1bias = psmall.tile([128, 8], F32, name="fc1bias")
    bfc2row = psmall.tile([1, C], BF16, name="bfc2row")
    ones1 = psmall.tile([1, TK], BF16, name="ones1")
    lbs = psmall.tile([128, 2], F32, name="lbs")
    id_b = psmall.tile([128, 128], BF16, name="id_b")
    mv1 = psmall.tile([TK, NTOK, 2], F32, name="mv1")
    rstd1 = psmall.tile([TK, NTOK], F32, name="rstd1")
    std1 = psmall.tile([TK, NTOK], F32, name="std1")
    mv2 = psmall.tile([TK, NTOK, 2], F32, name="mv2")
    rstd2 = psmall.tile([TK, NTOK], F32, name="rstd2")
    std2 = psmall.tile([TK, NTOK], F32, name="std2")
    eps_t = psmall.tile([TK, 1], F32, name="eps_t")
    nc.vector.memset(eps_t, EPS)
    nc.vector.memset(ones1, 1.0)

    # ---------------- phase A: loads + early weight conversion --------------
    stg_ctx = ExitStack()
    stg = stg_ctx.enter_context(tc.tile_pool(name="stg", bufs=1))
    wqkv_f = stg.tile([128, 2, 3 * C], F32, name="wqkv_f")
    wproj_f = stg.tile([128, 2, C], F32, name="wproj_f")
    lw_f = stg.tile([128, 2, 9], F32, name="lw_f")
    bfc2_f = stg.tile([1, C], F32, name="bfc2_f")

    # x on the sync queue (LN1-critical), weights on the scalar queue
    xsrc = x_d.rearrange("(i p) c -> p i c", p=TK)
    for j in range(4):
        nc.sync.dma_start(xtok[:, 7 * j:7 * j + 7, :], xsrc[:, 7 * j:7 * j + 7, :])
    nc.scalar.dma_start(wqkv_f, d["wqkv_d"].rearrange("(a p) n -> p a n", p=128))
    nc.scalar.dma_start(wproj_f, d["wproj_d"].rearrange("(a p) n -> p a n", p=128))
    nc.scalar.dma_start(qkvbias, d["qkvb_d"])
    nc.scalar.dma_start(fc1bias, d["fc1b_d"])
    nc.scalar.dma_start(bfc2_f, d["bfc2_d"])
    for bi in range(2):
        nc.scalar.dma_start(lw_f[:, bi, :], lw_d[bi])
        nc.scalar.dma_start(lbs[:, bi:bi + 1], lb_d[bi].unsqueeze(1))

    nc.vector.tensor_copy(wqkv_b, wqkv_f)
    nc.gpsimd.tensor_copy(lwb, lw_f)
    id_f = stg.tile([128, 128], F32, name="id_f")
    make_identity(nc, id_f)
    nc.gpsimd.tensor_copy(id_b, id_f)
    nc.vector.tensor_copy(wproj_b, wproj_f)
    nc.vector.tensor_copy(bfc2row, bfc2_f)
    stg_ctx.close()

    # ---------------- LN (token-major) helper ----------------
    def ln_stats(src, mv, i):
        st = lnp.tile([TK, 6], F32, name="bnst", tag="bnst")
        nc.vector.bn_stats(st, src[:, i, :])
        nc.vector.bn_aggr(mv[:, i, :], st)

    def ln_finish_group(mv, stdt, rstd, g):
        s = slice(4 * g, 4 * g + 4)
        nc.scalar.activation(stdt[:, s], mv[:, s, 1], AF.Sqrt, bias=eps_t)
        nc.vector.reciprocal(rstd[:, s], stdt[:, s])

    def ln_group(src, mv, rstd, dst, tpp, g):
        """ts on Pool (SBUF-only), transpose on PE, evict on ACT."""
        lnt = []
        for j in range(4):
            i = 4 * g + j
            lt = lnp.tile([TK, C], BF16, name="lnt", tag="lnt")
            nc.gpsimd.tensor_scalar(
                out=lt, in0=src[:, i, :],
                scalar1=mv[:, i, 0:1], scalar2=rstd[:, i:i + 1],
                op0=OP.subtract, op1=OP.mult)
            lnt.append(lt)
        for c in range(2):
            tp = tpp.tile([128, 4, TK], BF16, name="lntp", tag="lntp")
            for j in range(4):
                nc.tensor.transpose(tp[:, j, :],
                                    lnt[j][:, 128 * c:128 * (c + 1)],
                                    id_b[0:TK, 0:TK])
            nc.scalar.copy(
                dst[:, c, 4 * TK * g:4 * TK * (g + 1)].rearrange(
                    "p (j x) -> p j x", j=4), tp)

    # ---------------- attention-lifetime tensors ----------------
    lnp = ctx.enter_context(tc.tile_pool(name="lnp", bufs=8))
    actx = ExitStack()
    attn_pool = actx.enter_context(tc.tile_pool(name="attn_pool", bufs=1))
    qc = [attn_pool.tile([128, T], BF16, name=f"qc{b}") for b in range(2)]
    kc = [attn_pool.tile([128, T], BF16, name=f"kc{b}") for b in range(2)]
    vc = [attn_pool.tile([128, T], BF16, name=f"vc{b}") for b in range(2)]
    vtb = [attn_pool.tile([MC, NW, 4, 4 * 33], DT8, name=f"vtb{b}") for b in range(2)]
    lepe_sb = [attn_pool.tile([128, T], BF16, name=f"lepe{b}") for b in range(2)]
    dg = [attn_pool.tile([128, 9, 128], BF16, name=f"diag{b}") for b in range(2)]
    atok = attn_pool.tile([TK, 64, 128], BF16, name="atok")
    att_cm = attn_pool.tile([128, 2, T], DT8, name="att_cm")
    ln1_cm = attn_pool.tile([128, 2, T], DT8, name="ln1_cm")
    # static ones columns for the softmax denominator
    for bi in range(2):
        nc.vector.memset(
            vtb[bi].rearrange("p w m (h e) -> p w m h e", e=33)[:, :, :, :, 32:33],
            1.0)

    def emit_lepe_diag(bi):
        for tx in (-1, 0, 1):
            for ty in (-1, 0, 1):
                wi = ((ty + 1) * 3 + (tx + 1)) if bi == 0 else ((tx + 1) * 3 + (ty + 1))
                nc.gpsimd.affine_select(
                    out=dg[bi][:, wi, :],
                    in_=lwb[:, bi, wi:wi + 1].broadcast_to([128, 128]),
                    compare_op=OP.is_equal, fill=0.0, base=0,
                    pattern=[[-1, 128]], channel_multiplier=1)

    emit_lepe_diag(1)
    emit_lepe_diag(0)

    # ---------------- phase B: LN1 ----------------
    with tc.tile_pool(name="tpp1", bufs=2, space="PSUM") as tpp1:
        for g in range(7):
            for j in range(4):
                ln_stats(xtok, mv1, 4 * g + j)
            ln_finish_group(mv1, std1, rstd1, g)
            ln_group(xtok, mv1, rstd1, ln1_cm, tpp1, g)

    # ================= phase C/D: per-window pipeline =======================
    TAPS = [(0, 0)] + [(tx, ty) for tx in (-1, 0, 1) for ty in (-1, 0, 1)
                       if (tx, ty) != (0, 0)]

    with tc.tile_pool(name="qkvps", bufs=2, space="PSUM") as qkvps, \
         tc.tile_pool(name="vtps", bufs=1, space="PSUM") as vtps, \
         tc.tile_pool(name="sps", bufs=2, space="PSUM") as sps, \
         tc.tile_pool(name="ops", bufs=1, space="PSUM") as ops, \
         tc.tile_pool(name="etp", bufs=2) as etp, \
         tc.tile_pool(name="rrp", bufs=4) as rrp:

        def emit_qkv(bi, ci, c0, dst, t, eng):
            if "q" in SKIP:
                return
            pt = qkvps.tile([128, 512], F32, name="qkvt", tag="qkvt")
            if NODR:
                rhs = _rhs_qkv(ln1_cm, bi, t)
                for kch in range(2):
                    nc.tensor.matmul(pt[:, 0:L],
                                     wqkv_b[:, kch, c0:c0 + 128],
                                     rhs[:, kch],
                                     start=(kch == 0), stop=(kch == 1))
            else:
                nc.tensor.matmul(pt[:, 0:L],
                                 wqkv_b[:, :, c0:c0 + 128],
                                 _rhs_qkv(ln1_cm, bi, t),
                                 start=True, stop=True, perf_mode=PM.DoubleRow)
            eng.tensor_scalar_add(
                dst[:, L * t:L * (t + 1)], pt[:, 0:L], qkvbias[:, ci:ci + 1])

        def emit_vt(bi, w, eng):
            if "v" in SKIP:
                return
            vt = vtps.tile([128, 512], BF16, name="vtt", tag="vtt")
            for m in range(4):
                nc.tensor.transpose(
                    vt[0:MC, 128 * m:128 * (m + 1)],
                    vc[bi][:, L * w + MC * m:L * w + MC * (m + 1)],
                    id_b)
            dstv = vtb[bi][:, w].rearrange("p m (h e) -> p m h e", e=33)
            eng.tensor_copy(
                dstv[:, :, :, 0:32],
                vt[0:MC, :].rearrange("p (m h e) -> p m h e", m=4, h=4))

        natt = [0]

        def emit_attT(bi, w, q):
            if "t" in SKIP:
                return
            k = bi * 32 + w * 4 + q
            ln_ = QCS[q]
            tp = vtps.tile([128, 512], BF16, name="atp", tag="vtt")
            nc.tensor.transpose(tp[:, 0:ln_], atok[0:ln_, k, :],
                                id_b[0:ln_, 0:ln_])
            if bi == 0:
                dstw = att_cm[:, 0, :].rearrange(
                    "p (h w) -> p w h", h=HH)[:, 7 * w + 2 * q:
                                              7 * w + 2 * q + ln_ // 56, :]
                lpw = lepe_sb[0][:, L * w + QCO[q]:L * w + QCO[q] + ln_
                                 ].rearrange("p (a b) -> p a b", b=HH)
                srcw = tp[:, 0:ln_].rearrange("p (a b) -> p a b", b=HH)
            else:
                dstw = att_cm[:, 1, L * w + QCO[q]:L * w + QCO[q] + ln_]
                lpw = lepe_sb[1][:, L * w + QCO[q]:L * w + QCO[q] + ln_]
                srcw = tp[:, 0:ln_]
            nc.vector.tensor_tensor(out=dstw, in0=srcw, in1=lpw, op=OP.add)
            natt[0] += 1

        def emit_lepe(bi, w, on_act):
            if "l" in SKIP:
                return
            lp = qkvps.tile([128, 512], F32, name="lpt", tag="qkvt")
            lpv = lp[:, 0:L].rearrange("p (x y) -> p x y", x=7)
            vcv = vc[bi][:, L * w:L * (w + 1)].rearrange("p (x y) -> p x y", x=7)
            for ti, (tx, ty) in enumerate(TAPS):
                wi = ((ty + 1) * 3 + (tx + 1)) if bi == 0 else ((tx + 1) * 3 + (ty + 1))
                xo0, xo1 = max(0, -tx), 7 - max(0, tx)
                yo0, yo1 = max(0, -ty), HH - max(0, ty)
                nc.tensor.matmul(
                    lpv[:, xo0:xo1, yo0:yo1], dg[bi][:, wi, :],
                    vcv[:, xo0 + tx:xo1 + tx, yo0 + ty:yo1 + ty],
                    start=(ti == 0), stop=(ti == 8))
            dst = lepe_sb[bi][:, L * w:L * (w + 1)]
            if on_act:
                nc.scalar.activation(dst, lp[:, 0:L], AF.Identity,
                                     bias=lbs[:, bi:bi + 1])
            else:
                nc.vector.tensor_scalar_add(dst, lp[:, 0:L], lbs[:, bi:bi + 1])

        def emit_opass(bi, w, et, qpair):
            """O matmuls + normalize for q-chunks (2*qpair, 2*qpair+1)."""
            if "o" in SKIP:
                return
            ot = ops.tile([TK, 2, 132], F32, name="ot", tag="ot")
            if NODR:
                for m in range(4):
                    for qq in range(2):
                        q = 2 * qpair + qq
                        for h in range(NH):
                            nc.tensor.matmul(
                                ot[0:QCS[q], qq, 33 * h:33 * (h + 1)],
                                et[:, m, h, QCO[q]:QCO[q] + QCS[q]],
                                vtb[bi][:, w, m, 33 * h:33 * (h + 1)],
                                start=(m == 0), stop=(m == 3))
            else:
                for mp in range(2):
                    for qq in range(2):
                        q = 2 * qpair + qq
                        for h in range(NH):
                            nc.tensor.matmul(
                                ot[0:QCS[q], qq, 33 * h:33 * (h + 1)],
                                et[:, 2 * mp:2 * mp + 2, h,
                                   QCO[q]:QCO[q] + QCS[q]],
                                vtb[bi][:, w, 2 * mp:2 * mp + 2,
                                        33 * h:33 * (h + 1)],
                                start=(mp == 0), stop=(mp == 1),
                                perf_mode=PM.DoubleRow)
            rr = rrp.tile([TK, 2, 4], F32, name="rr", tag="rr")
            nc.vector.reciprocal(
                rr, ot.rearrange("p a (h e) -> p a h e", e=33)[:, :, :, 32])
            for qq in range(2):
                q = 2 * qpair + qq
                k = bi * 32 + w * 4 + q
                nc.vector.tensor_tensor(
                    out=atok[0:QCS[q], k, :].rearrange("p (h e) -> p h e", h=4),
                    in0=ot.rearrange(
                        "p a (h e) -> p a h e", e=33)[0:QCS[q], qq, :, 0:32],
                    in1=rr[0:QCS[q], qq, :].unsqueeze(2).broadcast_to(
                        [QCS[q], 4, 32]),
                    op=OP.mult)

        pending = []   # deferred O pass-B closures
        done_q = []    # windows whose normalize is fully emitted

        def emit_slot(bi, w, slot_idx):
            if len(done_q) >= 2:
                b2, w2 = done_q.pop(0)
                for q in range(4):
                    emit_attT(b2, w2, q)
            # prep: this window's qkv columns, V-transpose, LePE
            e1 = nc.vector
            e2 = nc.vector
            emit_qkv(bi, 0 if bi == 0 else 1, (0 if bi == 0 else 1) * 128,
                     qc[bi], w, e1)
            emit_qkv(bi, 2 if bi == 0 else 3, (2 if bi == 0 else 3) * 128,
                     kc[bi], w, e2)
            emit_qkv(bi, 4 if bi == 0 else 5, (4 if bi == 0 else 5) * 128,
                     vc[bi], w, e1)
            emit_vt(bi, w, nc.vector)
            emit_lepe(bi, w, on_act=False)
            if "s" in SKIP:
                return
            et = etp.tile([MC, 4, 4, L], DT8, name="et", tag="et")
            for m in range(4):
                for hp in range(2):
                    sp = sps.tile([MC, 2, 512], F32, name="spst", tag="spst")
                    for hh in range(2):
                        h = 2 * hp + hh
                        nc.tensor.matmul(
                            sp[:, hh, 0:L],
                            kc[bi][32 * h:32 * (h + 1),
                                   L * w + MC * m:L * w + MC * (m + 1)],
                            qc[bi][32 * h:32 * (h + 1), L * w:L * (w + 1)],
                            start=True, stop=True, tile_position=(32 * h, 0))
                    nc.scalar.activation(
                        et[:, m, 2 * hp:2 * hp + 2, :],
                        sp[:, :, 0:L], AF.Exp, scale=SM_SCALE)
                if m == 0 and pending:
                    pending.pop(0)()
            emit_opass(bi, w, et, 0)

            def passb(bi=bi, w=w, et=et):
                emit_opass(bi, w, et, 1)
                done_q.append((bi, w))
            pending.append(passb)

        # b1 runs two slots ahead (b0's W-major QKV reads need full LN1)
        order = [(1, 0), (1, 1)]
        for w in range(6):
            order += [(0, w), (1, w + 2)]
        order += [(0, 6), (0, 7)]
        for si, (bi, w) in enumerate(order):
            emit_slot(bi, w, si)
        while pending:
            pending.pop(0)()
        for b2, w2 in done_q:
            for q in range(4):
                emit_attT(b2, w2, q)

    # ---------------- phase E: proj + residual + LN2 stats ------------------
    with tc.tile_pool(name="prps", bufs=3, space="PSUM") as prps:
        for i2 in range(NTOK // 2):
            pt = prps.tile([TK, 2, C], F32, name="prt", tag="prt")
            for half in range(2):
                i = 2 * i2 + half
                if NODR:
                    for kch in range(2):
                        nc.tensor.matmul(pt[:, half, :],
                                         att_cm[:, kch, TK * i:TK * (i + 1)],
                                         wproj_b[:, kch, :],
                                         start=(kch == 0), stop=(kch == 1))
                else:
                    nc.tensor.matmul(pt[:, half, :],
                                     att_cm[:, :, TK * i:TK * (i + 1)],
                                     wproj_b,
                                     start=True, stop=True,
                                     perf_mode=PM.DoubleRow)
            nc.vector.scalar_tensor_tensor(
                out=x2res[:, 2 * i2:2 * i2 + 2, :], in0=pt, scalar=1.0,
                in1=xtok[:, 2 * i2:2 * i2 + 2, :], op0=OP.mult, op1=OP.add)
            ln_stats(x2res, mv2, 2 * i2)
            ln_stats(x2res, mv2, 2 * i2 + 1)

    actx.close()

    # ---------------- phase F: LN2 + MLP ----------------
    with tc.tile_pool(name="mlp", bufs=1) as mlp:
        ln2_cm = mlp.tile([128, 2, T], DT8, name="ln2_cm")
        h_sb = mlp.tile([128, 8, T], DT8, name="h_sb")
        wfc1_b = mlp.tile([128, 2, 4 * C], DT8, name="wfc1_b")
        wfc2_b = mlp.tile([128, 8, C], DT8, name="wfc2_b")
        wfc1_f = mlp.tile([128, 2, 4 * C], F32, name="wfc1_f")
        wfc2_f = mlp.tile([128, 8, C], F32, name="wfc2_f")
        nc.scalar.dma_start(wfc1_f, d["wfc1_d"].rearrange("(a p) n -> p a n", p=128))
        nc.scalar.dma_start(wfc2_f, d["wfc2_d"].rearrange("(a p) n -> p a n", p=128))
        nc.vector.tensor_copy(wfc1_b, wfc1_f)
        nc.vector.tensor_copy(wfc2_b, wfc2_f)

        with tc.tile_pool(name="tpp2", bufs=2, space="PSUM") as tpp2, \
             tc.tile_pool(name="f1ps", bufs=2, space="PSUM") as f1ps, \
             tc.tile_pool(name="f2ps", bufs=2, space="PSUM") as f2ps, \
             tc.tile_pool(name="otp", bufs=4) as otp:
            for g in range(7):
                ln_finish_group(mv2, std2, rstd2, g)
                ln_group(x2res, mv2, rstd2, ln2_cm, tpp2, g)

            def emit_fc2(i2):
                pt = f2ps.tile([TK, 2, C], F32, name="f2t", tag="f2t")
                w2v = wfc2_b.rearrange("p (k j) n -> p k j n", j=2)
                for half in range(2):
                    i = 2 * i2 + half
                    if NODR:
                        for k8 in range(8):
                            nc.tensor.matmul(pt[:, half, :],
                                             h_sb[:, k8, TK * i:TK * (i + 1)],
                                             wfc2_b[:, k8, :],
                                             start=(k8 == 0), stop=False)
                    else:
                        for k4 in range(4):
                            nc.tensor.matmul(pt[:, half, :],
                                             h_sb[:, 2 * k4:2 * k4 + 2,
                                                  TK * i:TK * (i + 1)],
                                             w2v[:, k4, :, :],
                                             start=(k4 == 0), stop=False,
                                             perf_mode=PM.DoubleRow)
                    nc.tensor.matmul(pt[:, half, :], ones1, bfc2row,
                                     start=False, stop=True)
                ot = otp.tile([TK, 2, C], F32, name="ot", tag="ot")
                nc.vector.scalar_tensor_tensor(
                    out=ot, in0=pt, scalar=1.0,
                    in1=x2res[:, 2 * i2:2 * i2 + 2, :],
                    op0=OP.mult, op1=OP.add)
                eng = nc.sync if i2 % 2 == 0 else nc.scalar
                eng.dma_start(
                    out_d[2 * TK * i2:2 * TK * (i2 + 1), :].rearrange(
                        "(a p) c -> p a c", p=TK),
                    ot)

            done_fc2 = 0
            for t2 in range(NW // 2):
                for m8 in range(8):
                    pt = f1ps.tile([128, 2, 512], F32, name="f1t", tag="f1t")
                    for half in range(2):
                        t = 2 * t2 + half
                        if NODR:
                            for kch in range(2):
                                nc.tensor.matmul(
                                    pt[:, half, 0:L],
                                    wfc1_b[:, kch, 128 * m8:128 * (m8 + 1)],
                                    ln2_cm[:, kch, L * t:L * (t + 1)],
                                    start=(kch == 0), stop=(kch == 1))
                        else:
                            nc.tensor.matmul(pt[:, half, 0:L],
                                             wfc1_b[:, :, 128 * m8:128 * (m8 + 1)],
                                             ln2_cm[:, :, L * t:L * (t + 1)],
                                             start=True, stop=True,
                                             perf_mode=PM.DoubleRow)
                    nc.scalar.activation(
                        h_sb[:, m8, 2 * L * t2:2 * L * (t2 + 1)].rearrange(
                            "p (a x) -> p a x", a=2),
                        pt[:, :, 0:L],
                        AF.Gelu, bias=fc1bias[:, m8:m8 + 1])
                while (done_fc2 + 1) * 2 * TK <= 2 * L * (t2 + 1):
                    emit_fc2(done_fc2)
                    done_fc2 += 1
            while done_fc2 < NTOK // 2:
                emit_fc2(done_fc2)
                done_fc2 += 1


def kernel(**inputs):
    if "nc" not in _CACHE:
        _CACHE["nc"] = _build()
    nc = _CACHE["nc"]

    x = np.asarray(inputs["x"], dtype=np.float32)          # [8, 56, 56, 256]
    g1 = np.asarray(inputs["gamma1"], np.float32)
    be1 = np.asarray(inputs["beta1"], np.float32)
    g2 = np.asarray(inputs["gamma2"], np.float32)
    be2 = np.asarray(inputs["beta2"], np.float32)
    wqkv = np.asarray(inputs["w_qkv"], np.float32)
    wfc1 = np.asarray(inputs["w_fc1"], np.float32)
    bproj = np.asarray(inputs["b_proj"], np.float32)

    # gamma folds into the weights, beta into additive per-channel biases,
    # b_proj into x (the proj residual path adds it to every token).
    wqkv_f = g1[:, None] * wqkv
    qkv_bias_full = be1 @ wqkv                             # [768]
    qkv_bias = np.stack([qkv_bias_full[128 * ci:128 * (ci + 1)]
                         for ci in range(6)], axis=1)      # [128, 6]
    wfc1_f = g2[:, None] * wfc1
    fc1_bias_full = be2 @ wfc1 + np.asarray(inputs["b_fc1"], np.float32)
    fc1_bias = np.stack([fc1_bias_full[128 * m:128 * (m + 1)]
                         for m in range(8)], axis=1)       # [128, 8]

    base = {
        "w_qkv": np.ascontiguousarray(wqkv_f),
        "qkv_bias": np.ascontiguousarray(qkv_bias),
        "w_proj": np.asarray(inputs["w_proj"], np.float32),
        "w_fc1": np.ascontiguousarray(wfc1_f),
        "fc1_bias": np.ascontiguousarray(fc1_bias),
        "w_fc2": np.asarray(inputs["w_fc2"], np.float32),
        "b_fc2": np.asarray(inputs["b_fc2"], np.float32).reshape(1, C),
        "lepe_w0": np.asarray(inputs["lepe_w0"], np.float32).reshape(128, 9),
        "lepe_w1": np.asarray(inputs["lepe_w1"], np.float32).reshape(128, 9),
        "lepe_b0": np.asarray(inputs["lepe_b0"], np.float32),
        "lepe_b1": np.asarray(inputs["lepe_b1"], np.float32),
    }
    xb = x + bproj[None, None, None, :]
    in_maps = [{**base, "x": np.ascontiguousarray(xb[i].reshape(T, C))}
               for i in range(B)]
    import os
    trace = bool(int(os.environ.get("BASS_KERNEL_TRACE", "0")))
    res = run_bass_kernel_spmd(nc, in_maps, core_ids=list(range(B)), trace=trace)
    _CACHE["last_results"] = res
    out = np.stack([res.results[i]["out"] for i in range(B)])
    return out.reshape(B, HH, WW, C)


if __name__ == "__main__":
    rng = np.random.default_rng(0)
    ins = {
        "x": rng.standard_normal((B, HH, WW, C), dtype=np.float32),
        "gamma1": np.ones(C, np.float32) + 0.1 * rng.standard_normal(C).astype(np.float32),
        "beta1": 0.1 * rng.standard_normal(C).astype(np.float32),
        "w_qkv": rng.standard_normal((C, 3 * C), dtype=np.float32) * 0.02,
        "lepe_w0": rng.standard_normal((128, 1, 3, 3), dtype=np.float32) * 0.02,
        "lepe_b0": np.zeros(128, np.float32),
        "lepe_w1": rng.standard_normal((128, 1, 3, 3), dtype=np.float32) * 0.02,
        "lepe_b1": np.zeros(128, np.float32),
        "w_proj": rng.standard_normal((C, C), dtype=np.float32) * 0.02,
        "b_proj": np.zeros(C, np.float32),
        "gamma2": np.ones(C, np.float32), "beta2": np.zeros(C, np.float32),
        "w_fc1": rng.standard_normal((C, 4 * C), dtype=np.float32) * 0.02,
        "b_fc1": np.zeros(4 * C, np.float32),
        "w_fc2": rng.standard_normal((4 * C, C), dtype=np.float32) * 0.02,
        "b_fc2": np.zeros(C, np.float32),
    }
    o = kernel(**ins)
    print("ran:", o.shape, o.dtype, float(np.abs(o).max()))
